# revision 1
# baseline (speedup 1.0000x reference)
"""Self-contained Trainium2 Bass kernel for nn_DualGATv2 (3-layer GATv2 + MLP).

Sharding: nodes are degree-sorted and snake-dealt across 8 NeuronCores
(graph parallel). Each core owns 6272 table rows (6250 real nodes + 22 pads)
and processes the edges whose *destination* lies in its shard; small weights
are replicated. Projected source features live in a bf16 table per layer
(layer 0 computed fully on every core since x is replicated; layers 1-2 via
AllGather). Per-edge features are fetched with gpsimd dma_gather (int16
indices; two gathers per destination block from overlapping lo/hi windows of
the table to cover >32k rows). Scatter-softmax/scatter-add become dense
per-partition ops: each destination node owns one SBUF partition of its
block, its (band-padded) incoming edges occupy free-dim slots, and an
additive -1e30 mask neutralizes pad slots.
"""
import sys
import numpy as np

sys.path.insert(0, '/opt/trn_rl_repo')

import concourse.bass as bass
import concourse.bacc as bacc
import concourse.tile as tile
from concourse import mybir, library_config
from concourse import bass_utils
from concourse._compat import cdiv

F32 = mybir.dt.float32
BF16 = mybir.dt.bfloat16
I16 = mybir.dt.int16
AL = mybir.AluOpType
ACTF = mybir.ActivationFunctionType
AX = mybir.AxisListType

NC = 8
P = 128
HID = 32
HEADS = 4
NEG_SLOPE = 0.2
LN_EPS = 1e-5
NEG_BIG = -1.0e30
IDX_WIN = 32768


# ----------------------------------------------------------------------------
# host-side preprocessing
# ----------------------------------------------------------------------------

def _prep(x, edge_index):
    x = np.asarray(x, dtype=np.float32)
    N = x.shape[0]
    src = np.asarray(edge_index[0], dtype=np.int64)
    dst = np.asarray(edge_index[1], dtype=np.int64)
    loop = np.arange(N, dtype=np.int64)
    src = np.concatenate([src, loop])
    dst = np.concatenate([dst, loop])

    deg = np.bincount(dst, minlength=N)

    order = np.argsort(-deg, kind='stable')
    ranks = np.arange(N)
    g, j = ranks // NC, ranks % NC
    core_of_rank = np.where(g % 2 == 0, j, NC - 1 - j)
    core = np.zeros(N, dtype=np.int64)
    core[order] = core_of_rank
    # position within core, in rank order
    pos = np.zeros(N, dtype=np.int64)
    cnt = np.zeros(NC, dtype=np.int64)
    for r in range(N):
        n = order[r]
        c = core[n]
        pos[n] = cnt[c]
        cnt[c] += 1

    NSH_REAL = cdiv(N, NC)
    NSH = cdiv(NSH_REAL + 1, P) * P       # ensure >= 1 pad row per core
    NBLK = NSH // P
    TAB = NC * NSH
    row = core * NSH + pos
    W_LO = min(IDX_WIN, TAB)
    HI_BASE = max(0, TAB - IDX_WIN)
    ZR_LO = NSH - 1                        # core 0's last pad row (< W_LO)
    ZR_HI = TAB - 1 - HI_BASE              # last core's last pad, hi-local

    e_order = np.argsort(dst, kind='stable')
    src_s = src[e_order]
    dst_s = dst[e_order]
    starts = np.searchsorted(dst_s, np.arange(N))
    ends = np.searchsorted(dst_s, np.arange(N) + 1)
    rs_all = row[src_s]

    cA = np.zeros(N, dtype=np.int64)
    cB = np.zeros(N, dtype=np.int64)
    edgeA = [None] * N
    edgeB = [None] * N
    for n in range(N):
        s, e = starts[n], ends[n]
        rs = rs_all[s:e]
        d = e - s
        forcedA = rs < HI_BASE
        forcedB = rs >= W_LO
        nAf = int(forcedA.sum())
        nBf = int(forcedB.sum())
        ca = min(max((d + 1) // 2, nAf), d - nBf)
        selA = forcedA.copy()
        nflexA = ca - nAf
        if nflexA > 0:
            fidx = np.nonzero(~forcedA & ~forcedB)[0]
            selA[fidx[:nflexA]] = True
        edgeA[n] = rs[selA]
        edgeB[n] = rs[~selA] - HI_BASE
        cA[n] = ca
        cB[n] = d - ca

    node_at = np.full((NC, NSH), -1, dtype=np.int64)
    node_at[core, pos] = np.arange(N)

    K_A = np.zeros(NBLK, dtype=np.int64)
    K_B = np.zeros(NBLK, dtype=np.int64)
    for b in range(NBLK):
        sl = node_at[:, b * P:(b + 1) * P].reshape(-1)
        sl = sl[sl >= 0]
        if len(sl):
            K_A[b] = cA[sl].max()
            K_B[b] = cB[sl].max()
    K_A = np.maximum(K_A, 1)
    K_B = np.maximum(K_B, 1)

    SUMKT = int((K_A + K_B).sum())
    IDXW = int(8 * SUMKT)
    idx_all = np.zeros((NC, P, IDXW), dtype=np.int16)
    mask_all = np.full((NC, P, SUMKT), NEG_BIG, dtype=np.float32)

    def wrap(flat):
        n = len(flat)
        S = cdiv(n, 16)
        a = np.zeros(16 * S, np.int16)
        a[:n] = flat
        return np.tile(a.reshape(S, 16).T, (8, 1))

    icol = 0
    mcol = 0
    for b in range(NBLK):
        ka, kb = int(K_A[b]), int(K_B[b])
        for c in range(NC):
            flatA = np.full(ka * P, ZR_LO, np.int64)
            flatB = np.full(kb * P, ZR_HI, np.int64)
            for p in range(P):
                n = node_at[c, b * P + p]
                if n < 0:
                    continue
                ea, eb = edgeA[n], edgeB[n]
                la, lb = len(ea), len(eb)
                if la:
                    flatA[np.arange(la) * P + p] = ea
                    mask_all[c, p, mcol:mcol + la] = 0.0
                if lb:
                    flatB[np.arange(lb) * P + p] = eb
                    mask_all[c, p, mcol + ka:mcol + ka + lb] = 0.0
            idx_all[c, :, icol:icol + 8 * ka] = wrap(flatA.astype(np.int16))
            idx_all[c, :, icol + 8 * ka:icol + 8 * (ka + kb)] = \
                wrap(flatB.astype(np.int16))
        icol += 8 * (ka + kb)
        mcol += ka + kb

    IND = x.shape[1]
    xT = np.zeros((IND, TAB), dtype=np.float32)
    xT[:, row] = x.T
    xT_own = np.ascontiguousarray(
        xT.reshape(IND, NC, NSH).transpose(1, 0, 2))   # [NC, IND, NSH]

    padmask = (np.arange(P) < (NSH_REAL - (NBLK - 1) * P)) \
        .astype(np.float32).reshape(P, 1)
    st = dict(N=N, NSH=NSH, NSH_REAL=NSH_REAL, NBLK=NBLK, TAB=TAB,
              W_LO=W_LO, HI_BASE=HI_BASE, K_A=K_A.tolist(),
              K_B=K_B.tolist(), SUMKT=SUMKT, IDXW=IDXW, IN_DIM=IND)
    return st, xT, xT_own, idx_all, mask_all, row, padmask


def _rep(v):
    v = np.asarray(v, dtype=np.float32).reshape(1, -1)
    return np.ascontiguousarray(np.tile(v, (P, 1)))


# ----------------------------------------------------------------------------
# kernel builder
# ----------------------------------------------------------------------------

def _build(st):
    import os
    STAGE = os.environ.get('STAGE', 'FULL')
    NSH, NBLK, TAB = st['NSH'], st['NBLK'], st['TAB']
    NSH_REAL = st['NSH_REAL']
    W_LO, HI_BASE = st['W_LO'], st['HI_BASE']
    K_A, K_B = st['K_A'], st['K_B']
    IDXW, SUMKT = st['IDXW'], st['SUMKT']
    IND = st['IN_DIM']
    NT = TAB // P
    PAD_P0 = NSH_REAL - (NBLK - 1) * P     # first pad partition in last block

    LCFG = [(HEADS, HID, HEADS * HID, IND),
            (HEADS, HID, HEADS * HID, HEADS * HID),
            (1, HID, HID, HEADS * HID)]

    nc = bacc.Bacc('TRN2', target_bir_lowering=False, debug=False,
                   enable_asserts=True, num_devices=NC,
                   num_swdge_queues=4)

    def ein(name, shape, dt=F32):
        return nc.dram_tensor(name, shape, dt, kind='ExternalInput')

    xT_d = ein('xT', [IND, TAB])
    xTo_d = ein('xT_own', [IND, NSH])
    idx_d = ein('idx_all', [P, IDXW], I16)
    pmask_d = ein('padmask', [P, 1])
    mask_d = ein('mask_all', [P, SUMKT])
    W01_d = [ein('W01_0', [IND, 256]), ein('W01_1', [128, 256]),
             ein('W01_2', [128, 64])]
    BL01_d = [ein('bl01_0', [P, 256]), ein('bl01_1', [P, 256]),
              ein('bl01_2', [P, 64])]
    ATT_d = [ein('att_0', [P, 128]), ein('att_1', [P, 128]),
             ein('att_2', [P, 32])]
    GG_d = [ein('g_0', [P, 128]), ein('g_1', [P, 128]), ein('g_2', [P, 32])]
    BE_d = [ein('be_0', [P, 128]), ein('be_1', [P, 128]), ein('be_2', [P, 32])]
    BO_d = [ein('bo_0', [P, 128]), ein('bo_1', [P, 128]), ein('bo_2', [P, 32])]
    cW1_d = ein('cW1', [32, 16])
    cb1_d = ein('cb1', [P, 16])
    cW2_d = ein('cW2', [16, 1])
    ident_d = ein('ident', [P, P])
    cb2_d = ein('cb2', [P, 1])
    out_d = nc.dram_tensor('out', [NSH], F32, kind='ExternalOutput')

    tabs = [nc.dram_tensor('table0', [TAB, 128], BF16, kind='Internal'),
            nc.dram_tensor('table1', [TAB, 128], BF16, kind='Internal',
                           addr_space='Shared'),
            nc.dram_tensor('table2', [TAB, 128], BF16, kind='Internal',
                           addr_space='Shared')]
    ag_in = [None,
             nc.dram_tensor('ag_in1', [NSH, 128], BF16, kind='Internal'),
             nc.dram_tensor('ag_in2', [NSH, 128], BF16, kind='Internal')]

    import contextlib
    with tile.TileContext(nc) as tc, contextlib.ExitStack() as ctx:
        cpool = ctx.enter_context(tc.tile_pool(name='consts', bufs=1))
        gpool = ctx.enter_context(tc.tile_pool(name='g', bufs=3))
        tpool = ctx.enter_context(tc.tile_pool(name='t', bufs=2))
        spool = ctx.enter_context(tc.tile_pool(name='small', bufs=3))
        npool = ctx.enter_context(tc.tile_pool(name='node', bufs=2))
        hpool = ctx.enter_context(tc.tile_pool(name='h', bufs=1))
        xpool = ctx.enter_context(tc.tile_pool(name='xt', bufs=6))
        stpool = ctx.enter_context(tc.tile_pool(name='stage', bufs=4))
        pspool = ctx.enter_context(tc.tile_pool(name='ps', bufs=3,
                                                space='PSUM'))
        ps2pool = ctx.enter_context(tc.tile_pool(name='ps2', bufs=2,
                                                 space='PSUM'))

        def load_const(dram, shape, dt=F32):
            t = cpool.tile(shape, dt, tag='c_' + dram.name,
                           name='c_' + dram.name)
            nc.sync.dma_start(out=t[:], in_=dram[:])
            return t

        ident = load_const(ident_d, [P, P])
        W01_s = [load_const(W01_d[l], list(W01_d[l].shape)) for l in range(3)]
        BL01_s = [load_const(BL01_d[l], list(BL01_d[l].shape)) for l in range(3)]
        GG_s = [load_const(GG_d[l], list(GG_d[l].shape)) for l in range(3)]
        BE_s = [load_const(BE_d[l], list(BE_d[l].shape)) for l in range(3)]
        BO_s = [load_const(BO_d[l], list(BO_d[l].shape)) for l in range(3)]
        cW1_s = load_const(cW1_d, [32, 16])
        cb1_s = load_const(cb1_d, [P, 16])
        cW2_s = load_const(cW2_d, [16, 1])
        cb2_s = load_const(cb2_d, [P, 1])
        attb = []
        for l in range(3):
            f = load_const(ATT_d[l], ATT_d[l].shape)
            t = cpool.tile(ATT_d[l].shape, BF16, tag=f'attb{l}',
                           name=f'attb{l}')
            nc.vector.tensor_copy(out=t[:], in_=f[:])
            attb.append(t)

        pmask_s = load_const(pmask_d, [P, 1])
        eps_t = cpool.tile([P, 1], F32, tag='eps', name='eps')
        nc.vector.memset(eps_t[:], float(LN_EPS))
        mask_s = cpool.tile([P, SUMKT], F32, tag='mask')
        nc.sync.dma_start(out=mask_s[:], in_=mask_d[:])

        h_res = [hpool.tile([P, NBLK * 128], F32, tag='h0', name='h0'),
                 hpool.tile([P, NBLK * 128], F32, tag='h1', name='h1'),
                 hpool.tile([P, NBLK * HID], F32, tag='h2', name='h2')]
        xr_res = [hpool.tile([P, NBLK * 128], BF16, tag='xr0', name='xr0'),
                  hpool.tile([P, NBLK * 128], BF16, tag='xr1', name='xr1'),
                  hpool.tile([P, NBLK * HID], BF16, tag='xr2', name='xr2')]
        out_sb = hpool.tile([P, NBLK], F32, tag='outsb')

        # ------------- layer-0 projections (x replicated => local) -------
        for t in range(NT):
            xt = xpool.tile([IND, P], F32, tag='xt')
            nc.scalar.dma_start(out=xt[:], in_=xT_d[:, t * P:(t + 1) * P])
            ps = pspool.tile([P, 256], F32, tag='psA')
            nc.tensor.matmul(out=ps[:, 0:128], lhsT=xt[:],
                             rhs=W01_s[0][:, 0:128], start=True, stop=True)
            stg = stpool.tile([P, 128], BF16, tag='stgA')
            nc.vector.tensor_tensor(out=stg[:], in0=ps[:, 0:128],
                                    in1=BL01_s[0][:, 0:128], op=AL.add)
            if t % NBLK == NBLK - 1:
                nc.vector.tensor_scalar_mul(stg[:], stg[:], pmask_s[:])
            nc.sync.dma_start(out=tabs[0][t * P:(t + 1) * P, :], in_=stg[:])
        for b in range(NBLK):
            xo = xpool.tile([IND, P], F32, tag='xo')
            nc.scalar.dma_start(out=xo[:], in_=xTo_d[:, b * P:(b + 1) * P])
            ps = pspool.tile([P, 256], F32, tag='psA')
            nc.tensor.matmul(out=ps[:, 0:128], lhsT=xo[:],
                             rhs=W01_s[0][:, 128:256], start=True, stop=True)
            nc.vector.tensor_tensor(out=xr_res[0][:, b * 128:(b + 1) * 128],
                                    in0=ps[:, 0:128],
                                    in1=BL01_s[0][:, 128:256], op=AL.add)

        # cumulative idx/mask offsets per block
        ic_of = []
        mc_of = []
        ic = mc = 0
        for b in range(NBLK):
            ic_of.append(ic)
            mc_of.append(mc)
            ic += 8 * (K_A[b] + K_B[b])
            mc += K_A[b] + K_B[b]

        qc = [0]

        def edge_block(l, b):
            H, DO, FE, _ = LCFG[l]
            ka, kb = K_A[b], K_B[b]
            kt = ka + kb
            ic, mc = ic_of[b], mc_of[b]
            GA = gpool.tile([P, ka, 128], BF16, tag='GA')
            GB = gpool.tile([P, kb, 128], BF16, tag='GB')
            idx_s = xpool.tile([P, 8 * kt], I16, tag='idxblk')
            nc.scalar.dma_start(out=idx_s[:], in_=idx_d[:, ic:ic + 8 * kt])
            ic = 0
            GMAX = 7  # 7*128 idxs -> 56+sem descriptors per engine packet
            for off in range(0, ka, GMAX):
                kk = min(GMAX, ka - off)
                nc.gpsimd.dma_gather(
                    GA[:, off:off + kk, :], tabs[l][0:W_LO, :],
                    idx_s[:, ic + 8 * off:ic + 8 * (off + kk)],
                    kk * P, kk * P, 128, queue_num=qc[0] % 4)
                qc[0] += 1
            for off in range(0, kb, GMAX):
                kk = min(GMAX, kb - off)
                nc.gpsimd.dma_gather(
                    GB[:, off:off + kk, :], tabs[l][HI_BASE:TAB, :],
                    idx_s[:, ic + 8 * (ka + off):ic + 8 * (ka + off + kk)],
                    kk * P, kk * P, 128, queue_num=qc[0] % 4)
                qc[0] += 1
            AGG = npool.tile([P, FE], F32, tag='AGG')
            T = tpool.tile([P, kt * FE], BF16, tag='T')
            T3 = T[:].rearrange('p (k f) -> p k f', k=kt)
            xr_col = xr_res[l][:, b * FE:(b + 1) * FE]
            nc.vector.tensor_tensor(
                out=T3[:, 0:ka, :], in0=GA[:, :, 0:FE],
                in1=xr_col.unsqueeze(1).to_broadcast([P, ka, FE]), op=AL.add)
            nc.vector.tensor_tensor(
                out=T3[:, ka:kt, :], in0=GB[:, :, 0:FE],
                in1=xr_col.unsqueeze(1).to_broadcast([P, kb, FE]), op=AL.add)
            nc.vector.scalar_tensor_tensor(
                out=T[:], in0=T[:], scalar=float(NEG_SLOPE), in1=T[:],
                op0=AL.mult, op1=AL.max)
            att_b = attb[l][:].unsqueeze(1).to_broadcast([P, kt, FE])
            nc.vector.tensor_tensor(out=T3, in0=T3, in1=att_b, op=AL.mult)
            # k-major logits: LG[p, k*H + h]
            LG = spool.tile([P, kt * H], F32, tag='LG')
            LG3 = LG[:].rearrange('p (k h) -> p k h', h=H)
            LG_hk = LG[:].rearrange('p (k h) -> p h k', h=H)
            T_khd = T[:].rearrange('p (k h d) -> p k h d', h=H, d=DO)
            nc.vector.tensor_reduce(out=LG[:], in_=T_khd, axis=AX.X,
                                    op=AL.add)
            mask_b = mask_s[:, mc:mc + kt].unsqueeze(2) \
                .to_broadcast([P, kt, H])
            nc.vector.tensor_tensor(out=LG3, in0=LG3, in1=mask_b, op=AL.add)
            M = spool.tile([P, H], F32, tag='M')
            nc.vector.tensor_reduce(out=M[:], in_=LG_hk, axis=AX.X,
                                    op=AL.max)
            nc.vector.tensor_tensor(
                out=LG3, in0=LG3,
                in1=M[:].unsqueeze(1).to_broadcast([P, kt, H]),
                op=AL.subtract)
            nc.scalar.activation(out=LG[:], in_=LG[:], func=ACTF.Exp)
            DN = spool.tile([P, H], F32, tag='DN')
            nc.vector.tensor_reduce(out=DN[:], in_=LG_hk, axis=AX.X,
                                    op=AL.add)
            R = spool.tile([P, H], F32, tag='R')
            nc.vector.reciprocal(R[:], DN[:])
            Abf = spool.tile([P, kt * H], BF16, tag='Abf')
            nc.scalar.copy(out=Abf[:], in_=LG[:])
            # weighted messages into T (reused), layout (h, d, k)
            W_out = T[:].rearrange('p (h d k) -> p h d k', h=H, d=DO)
            A3 = Abf[:].rearrange('p (k h) -> p h k', h=H)
            nc.vector.tensor_tensor(
                out=W_out[:, :, :, 0:ka],
                in0=GA[:, :, 0:FE].rearrange('p k (h d) -> p h d k', h=H),
                in1=A3[:, :, 0:ka].unsqueeze(2).to_broadcast([P, H, DO, ka]),
                op=AL.mult)
            nc.vector.tensor_tensor(
                out=W_out[:, :, :, ka:kt],
                in0=GB[:, :, 0:FE].rearrange('p k (h d) -> p h d k', h=H),
                in1=A3[:, :, ka:kt].unsqueeze(2).to_broadcast([P, H, DO, kb]),
                op=AL.mult)
            nc.vector.tensor_reduce(out=AGG[:], in_=W_out, axis=AX.X,
                                    op=AL.add)
            AGG3 = AGG[:].rearrange('p (h d) -> p h d', h=H)
            nc.vector.tensor_tensor(
                out=AGG3, in0=AGG3,
                in1=R[:].unsqueeze(2).to_broadcast([P, H, DO]), op=AL.mult)
            nc.vector.tensor_tensor(out=AGG[:], in0=AGG[:], in1=BO_s[l][:],
                                    op=AL.add)
            return AGG

        def node_tail(l, b, AGG):
            H, DO, FE, _ = LCFG[l]
            if NT == 'off':
                nc.vector.tensor_copy(out=h_res[l][:, b * FE:(b + 1) * FE],
                                      in_=AGG[:])
                return
            SM = npool.tile([P, 1], F32, tag='SM')
            nc.vector.tensor_reduce(out=SM[:], in_=AGG[:], axis=AX.X,
                                    op=AL.add)
            MU = npool.tile([P, 1], F32, tag='MU')
            nc.vector.tensor_scalar_mul(MU[:], SM[:], 1.0 / FE)
            SQJ = npool.tile([P, FE], F32, tag='SQJ')
            SSQ = npool.tile([P, 1], F32, tag='SSQ')
            nc.scalar.activation(out=SQJ[:], in_=AGG[:], func=ACTF.Square,
                                 accum_out=SSQ[:])
            MU2 = npool.tile([P, 1], F32, tag='MU2')
            nc.scalar.activation(out=MU2[:], in_=MU[:], func=ACTF.Square)
            VAR = npool.tile([P, 1], F32, tag='VAR')
            nc.vector.scalar_tensor_tensor(
                out=VAR[:], in0=SSQ[:], scalar=1.0 / FE, in1=MU2[:],
                op0=AL.mult, op1=AL.subtract)
            SD = npool.tile([P, 1], F32, tag='SD')
            nc.scalar.activation(out=SD[:], in_=VAR[:], func=ACTF.Sqrt,
                                 bias=eps_t[:])
            IV = npool.tile([P, 1], F32, tag='IV')
            nc.vector.reciprocal(IV[:], SD[:])
            XH = npool.tile([P, FE], F32, tag='XH')
            nc.vector.scalar_tensor_tensor(
                out=XH[:], in0=AGG[:], scalar=MU[:],
                in1=IV[:].to_broadcast([P, FE]), op0=AL.subtract,
                op1=AL.mult)
            nc.vector.tensor_tensor(out=XH[:], in0=XH[:], in1=GG_s[l][:],
                                    op=AL.mult)
            nc.vector.tensor_tensor(out=XH[:], in0=XH[:], in1=BE_s[l][:],
                                    op=AL.add)
            MN = npool.tile([P, FE], F32, tag='MN')
            nc.vector.tensor_scalar_min(MN[:], XH[:], 0.0)
            EX = npool.tile([P, FE], F32, tag='EX')
            nc.scalar.activation(out=EX[:], in_=MN[:], func=ACTF.Exp)
            RL = npool.tile([P, FE], F32, tag='RL')
            nc.vector.tensor_scalar_max(RL[:], XH[:], 0.0)
            hcol = h_res[l][:, b * FE:(b + 1) * FE]
            if l == 1:
                TMP = npool.tile([P, FE], F32, tag='TMP')
                nc.vector.scalar_tensor_tensor(
                    out=TMP[:], in0=EX[:], scalar=-1.0, in1=RL[:],
                    op0=AL.add, op1=AL.add)
                nc.vector.tensor_tensor(
                    out=hcol, in0=TMP[:],
                    in1=h_res[0][:, b * FE:(b + 1) * FE], op=AL.add)
            else:
                nc.vector.scalar_tensor_tensor(
                    out=hcol, in0=EX[:], scalar=-1.0, in1=RL[:],
                    op0=AL.add, op1=AL.add)

        def phase_b(l):
            # projections for layer l (1 or 2) from h_res[l-1]; xl rows to
            # ag_in[l], xr into xr_res[l]
            _, _, FE, _ = LCFG[l]
            C0 = 128 if l == 1 else 32
            hsrc = h_res[l - 1]
            for b in range(NBLK):
                psT = ps2pool.tile([P, P], F32, tag='psT')
                nc.tensor.transpose(out=psT[:],
                                    in_=hsrc[:, b * 128:(b + 1) * 128],
                                    identity=ident[:])
                hT = stpool.tile([P, P], F32, tag='hT')
                nc.scalar.copy(out=hT[:], in_=psT[:])
                ps = pspool.tile([P, 256], F32, tag='psA')
                wcols = 256 if l == 1 else 64
                nc.tensor.matmul(out=ps[:, 0:wcols], lhsT=hT[:],
                                 rhs=W01_s[l][:], start=True, stop=True)
                stg = stpool.tile([P, 128], BF16, tag='stgB')
                if l == 1:
                    nc.vector.tensor_tensor(out=stg[:], in0=ps[:, 0:128],
                                            in1=BL01_s[1][:, 0:128],
                                            op=AL.add)
                else:
                    nc.vector.memset(stg[:], 0)
                    nc.vector.tensor_tensor(out=stg[:, 0:32],
                                            in0=ps[:, 0:32],
                                            in1=BL01_s[2][:, 0:32],
                                            op=AL.add)
                if b == NBLK - 1:
                    nc.vector.tensor_scalar_mul(stg[:], stg[:], pmask_s[:])
                nc.sync.dma_start(out=ag_in[l][b * P:(b + 1) * P, :],
                                  in_=stg[:])
                nc.vector.tensor_tensor(
                    out=xr_res[l][:, b * FE:(b + 1) * FE],
                    in0=ps[:, C0:C0 + FE], in1=BL01_s[l][:, C0:C0 + FE],
                    op=AL.add)

        # ---------------- layer 0 ----------------
        for b in range(NBLK):
            node_tail(0, b, edge_block(0, b))
        # ---------------- layer 1 ----------------
        phase_b(1)
        nc.gpsimd.collective_compute(
            'AllGather', AL.bypass, replica_groups=[list(range(NC))],
            ins=[ag_in[1][:]], outs=[tabs[1][:]])
        for b in range(NBLK):
            node_tail(1, b, edge_block(1, b))
        # ---------------- layer 2 ----------------
        phase_b(2)
        nc.gpsimd.collective_compute(
            'AllGather', AL.bypass, replica_groups=[list(range(NC))],
            ins=[ag_in[2][:]], outs=[tabs[2][:]])
        for b in range(NBLK):
            node_tail(2, b, edge_block(2, b))
        # ---------------- MLP head ----------------
        for b in range(NBLK):
            psT = ps2pool.tile([P, P], F32, tag='psT')
            nc.tensor.transpose(out=psT[:HID, :],
                                in_=h_res[2][:, b * HID:(b + 1) * HID],
                                identity=ident[:])
            h2T = stpool.tile([HID, P], F32, tag='h2T')
            nc.scalar.copy(out=h2T[:], in_=psT[:HID, :])
            ps1 = pspool.tile([P, 16], F32, tag='psM')
            nc.tensor.matmul(out=ps1[:], lhsT=h2T[:], rhs=cW1_s[:],
                             start=True, stop=True)
            C1 = npool.tile([P, 16], F32, tag='C1')
            nc.vector.tensor_tensor(out=C1[:], in0=ps1[:], in1=cb1_s[:],
                                    op=AL.add)
            MN1 = npool.tile([P, 16], F32, tag='MN1')
            nc.vector.tensor_scalar_min(MN1[:], C1[:], 0.0)
            EX1 = npool.tile([P, 16], F32, tag='EX1')
            nc.scalar.activation(out=EX1[:], in_=MN1[:], func=ACTF.Exp)
            RL1 = npool.tile([P, 16], F32, tag='RL1')
            nc.vector.tensor_scalar_max(RL1[:], C1[:], 0.0)
            E1 = npool.tile([P, 16], F32, tag='E1')
            nc.vector.scalar_tensor_tensor(
                out=E1[:], in0=EX1[:], scalar=-1.0, in1=RL1[:],
                op0=AL.add, op1=AL.add)
            psT2 = ps2pool.tile([P, P], F32, tag='psT')
            nc.tensor.transpose(out=psT2[:16, :], in_=E1[:],
                                identity=ident[:])
            c1T = stpool.tile([16, P], F32, tag='c1T')
            nc.scalar.copy(out=c1T[:], in_=psT2[:16, :])
            ps2 = pspool.tile([P, 16], F32, tag='psM')
            nc.tensor.matmul(out=ps2[:, 0:1], lhsT=c1T[:], rhs=cW2_s[:],
                             start=True, stop=True)
            nc.vector.tensor_tensor(out=out_sb[:, b:b + 1],
                                    in0=ps2[:, 0:1], in1=cb2_s[:],
                                    op=AL.add)
        nc.sync.dma_start(out=out_d[:].rearrange('(b p) -> p b', p=P),
                          in_=out_sb[:])

    nc.compile()
    return nc


# ----------------------------------------------------------------------------
# entry point
# ----------------------------------------------------------------------------

def _make_in_maps(st, inputs, xT, xT_own, idx_all, mask_all, pm):
    Wl0 = np.asarray(inputs['Wl0'], np.float32)
    Wr0 = np.asarray(inputs['Wr0'], np.float32)
    Wl1 = np.asarray(inputs['Wl1'], np.float32)
    Wr1 = np.asarray(inputs['Wr1'], np.float32)
    Wl2 = np.asarray(inputs['Wl2'], np.float32)
    Wr2 = np.asarray(inputs['Wr2'], np.float32)
    shared = {
        'xT': xT,
        'W01_0': np.ascontiguousarray(np.concatenate([Wl0, Wr0], axis=1)),
        'W01_1': np.ascontiguousarray(np.concatenate([Wl1, Wr1], axis=1)),
        'W01_2': np.ascontiguousarray(np.concatenate([Wl2, Wr2], axis=1)),
        'bl01_0': _rep(np.concatenate([inputs['bl0'], inputs['br0']])),
        'bl01_1': _rep(np.concatenate([inputs['bl1'], inputs['br1']])),
        'bl01_2': _rep(np.concatenate([inputs['bl2'], inputs['br2']])),
        'att_0': _rep(np.asarray(inputs['att0']).reshape(-1)),
        'att_1': _rep(np.asarray(inputs['att1']).reshape(-1)),
        'att_2': _rep(np.asarray(inputs['att2']).reshape(-1)),
        'g_0': _rep(inputs['g0']), 'g_1': _rep(inputs['g1']),
        'g_2': _rep(inputs['g2']),
        'be_0': _rep(inputs['be0']), 'be_1': _rep(inputs['be1']),
        'be_2': _rep(inputs['be2']),
        'bo_0': _rep(inputs['bo0']), 'bo_1': _rep(inputs['bo1']),
        'bo_2': _rep(inputs['bo2']),
        'cW1': np.asarray(inputs['cW1'], np.float32),
        'cb1': _rep(inputs['cb1']),
        'cW2': np.asarray(inputs['cW2'], np.float32),
        'ident': np.eye(P, dtype=np.float32),
        'cb2': _rep(inputs['cb2']),
    }
    in_maps = []
    for c in range(NC):
        m = dict(shared)
        m['padmask'] = pm
        m['xT_own'] = xT_own[c]
        m['idx_all'] = idx_all[c]
        m['mask_all'] = mask_all[c]
        in_maps.append(m)
    return in_maps


_CACHE = {}


def _run_sim(nc, in_maps):
    from concourse.bass_interp import MultiCoreSim
    sim = MultiCoreSim(nc, num_cores=NC, trace=False,
                       require_finite=False, require_nnan=False)
    cores = list(sim.cores.values())
    for c in range(NC):
        for k, v in in_maps[c].items():
            cores[c].tensor(k)[:] = v
    sim.simulate(check_with_hw=False)
    return [{'out': np.array(cores[c].tensor('out'))} for c in range(NC)]


def kernel(trace=False, backend='hw', **inputs):
    x = np.asarray(inputs['x'], np.float32)
    (st, xT, xT_own, idx_all, mask_all, row,
     padmask) = _prep(x, inputs['edge_index'])
    key = (x.shape, np.asarray(inputs['edge_index']).shape)
    skey = str(sorted(st.items()))
    if skey not in _CACHE:
        _CACHE[skey] = _build(st)
    nc = _CACHE[skey]
    in_maps = _make_in_maps(st, inputs, xT, xT_own, idx_all, mask_all,
                            padmask)
    if backend == 'sim':
        results = _run_sim(nc, in_maps)
        res = None
    else:
        res = bass_utils.run_bass_kernel_spmd(
            nc, in_maps, core_ids=list(range(NC)), trace=trace)
        results = res.results
    cat = np.concatenate([results[c]['out'] for c in range(NC)])
    out = cat[row]
    if trace:
        kernel.last_results = res
    return out.astype(np.float32)

